# revision 14
# baseline (speedup 1.0000x reference)
"""Trainium2 Bass kernel for sparse-attention grouped-query pooling.

Reference computation (B=1024, U=32, D=1024):
    k = input_h @ key_weight.T + key_bias                  # [B, D]
    q = einsum('bud,udq->buq', domain_aware_h, query_weight)
    scores = sigmoid(einsum('buq,bq->bu', q, k) / sqrt(D)) * (arange(U) <= t)
    w = scores / scores.sum(-1, keepdims=True)
    out = (domain_aware_h * w[..., None]).sum(1)           # [B, D]
    returns (out, scores)

Only units 0..t contribute to either output (masked scores are exactly
zero), so the kernel computes just the n_active = t+1 active units.

Distribution over 8 NeuronCores (tensor-parallel by unit, with the
remainder unit split by batch):
  - npc = n_active // 8 full units per core (units npc*c + i)
  - nrem = n_active % 8 remainder units, each split across cores by
    batch chunk (core c handles batch rows [c*128, (c+1)*128))
  - score dot uses the re-association  score[b,u] = dah[b,u,:] . m_u[b,:]
    with m_u = k @ qw[u].T, so no on-device transposes are needed
    (weights are pre-transposed and pre-cast to bf16 on the host).
  - kT ([qd, b] layout of k) is computed fully on every core (34us of
    TensorE, cheaper than an AllGather which measured ~60us wall);
    per-unit scores are exchanged with two small AllToAlls (split by
    batch-tile half so the first one hides under the main loop), then
    each core does the pooling for its own batch chunk.
  - pooling runs on the TensorEngine as an accumulated diag(s_u) @ dah
    matmul with a final 1/sum(s) normalization, so the tail is short.
"""

import os
import sys

if "/opt/trn_rl_repo" not in sys.path:
    sys.path.insert(0, "/opt/trn_rl_repo")

import numpy as np
import ml_dtypes

import concourse.bass as bass
import concourse.mybir as mybir
import concourse.tile as tile
from concourse import bacc
from concourse.bass_utils import run_bass_kernel_spmd

BF16 = ml_dtypes.bfloat16
B, U, D = 1024, 32, 1024
NCORES = 8
PB = B // NCORES  # batch rows per core = 128
F32 = mybir.dt.float32
BF = mybir.dt.bfloat16

_BUILD_CACHE: dict[int, object] = {}


def _build(n_active: int):
    """Build + compile the SPMD graph for n_active active units."""
    kt_local = os.environ.get("BASS_KT_LOCAL", "1") == "1"
    split_a2a = os.environ.get("BASS_SPLIT_A2A", "1") == "1"
    npc = n_active // NCORES        # full units per core
    nrem = n_active % NCORES        # remainder units (batch-split)

    nc = bacc.Bacc(None, target_bir_lowering=False, debug=False)

    # ---- parameters (per-core data, same graph on all cores) ----
    ihT = nc.declare_dram_parameter("ihT", [D, PB], BF, isOutput=False)
    if kt_local:
        ihTf = nc.declare_dram_parameter("ihTf", [D, B], BF, isOutput=False)
    kwT = nc.declare_dram_parameter("kwT", [D, D], BF, isOutput=False)
    biasr = nc.declare_dram_parameter("biasr", [128, 8], F32, isOutput=False)
    ident = nc.declare_dram_parameter("ident", [128, 128], BF, isOutput=False)
    zeros4k = nc.declare_dram_parameter("zeros4k", [128, 8], F32, isOutput=False)
    if npc > 0:
        qwf = nc.declare_dram_parameter("qwf", [npc, D, D], BF, isOutput=False)
    qwr = [
        nc.declare_dram_parameter(f"qwr{r}", [D, D], BF, isOutput=False)
        for r in range(nrem)
    ]
    dahs = nc.declare_dram_parameter("dahs", [n_active, PB, D], BF, isOutput=False)
    dahp = nc.declare_dram_parameter("dahp", [n_active, PB, D], BF, isOutput=False)

    out_p = nc.declare_dram_parameter("out", [PB, D], F32, isOutput=True)
    scores_p = nc.declare_dram_parameter("scores", [PB, n_active], F32, isOutput=True)

    with tile.TileContext(nc) as tc:
        with (
            tc.tile_pool(name="sb", bufs=1) as sb,
            tc.tile_pool(name="ps", bufs=1, space="PSUM") as ps,
            tc.tile_pool(name="dram", bufs=1, space="DRAM") as dram,
        ):
            # ---- collective bounce buffers ----
            if not kt_local:
                kt_in = dram.tile([D, PB], BF)
                kt_all = dram.tile([NCORES, D, PB], BF, addr_space="Shared")
            n_a2a = 2 if split_a2a else 1
            if npc > 0:
                a2a_in = [
                    dram.tile([NCORES * npc * PB], F32, name=f"a2a_in{h}")
                    for h in range(n_a2a)
                ]
                a2a_out = [
                    dram.tile([NCORES * npc * PB], F32, name=f"a2a_out{h}")
                    for h in range(n_a2a)
                ]

            # ---- priority loads: what the kT phase needs, chunked by dc ----
            kw_sb = []
            kwT_r = kwT.rearrange("(dc p) j -> dc p j", p=128)
            for dc in range(8):
                t_ = sb.tile([128, D], BF, name=f"kw_sb{dc}", tag=f"kw{dc}")
                nc.sync.dma_start(t_[:], kwT_r[dc])
                kw_sb.append(t_)
            ih_sb = sb.tile([128, 8, PB], BF)
            nc.sync.dma_start(ih_sb[:], ihT.rearrange("(dc p) b -> p dc b", p=128))
            if kt_local:
                ihf_sb = []
                ihTf_r = ihTf.rearrange("(dc p) b -> dc p b", p=128)
                for dc in range(8):
                    t_ = sb.tile([128, B], BF, name=f"ihf_sb{dc}", tag=f"ihf{dc}")
                    nc.sync.dma_start(t_[:], ihTf_r[dc])
                    ihf_sb.append(t_)
            bias_sb = sb.tile([128, 8], F32)
            nc.sync.dma_start(bias_sb[:], biasr[:])
            ident_sb = sb.tile([128, 128], BF)
            nc.sync.dma_start(ident_sb[:], ident[:])
            qwr_sb = []
            for r in range(nrem):
                t_ = sb.tile([128, 8, D], BF, name=f"qwr_sb{r}", tag=f"qwr{r}")
                nc.sync.dma_start(t_[:], qwr[r].rearrange("(qc p) d -> p qc d", p=128))
                qwr_sb.append(t_)

            # ---- kT-own phase (for the remainder units): kT[:, own chunk] ----
            kt_ps = ps.tile([128, 8, PB], F32, tag="acc", bufs=4)
            for jc in range(8):
                for dc in range(8):
                    nc.tensor.matmul(
                        kt_ps[:, jc, :],
                        kw_sb[dc][:, jc * 128 : (jc + 1) * 128],
                        ih_sb[:, dc, :],
                        start=(dc == 0),
                        stop=(dc == 7),
                    )
            kt_bf = sb.tile([128, 8, PB], BF)
            for jc in range(8):
                nc.vector.tensor_scalar(
                    kt_bf[:, jc, :],
                    kt_ps[:, jc, :],
                    bias_sb[:, jc : jc + 1],
                    None,
                    op0=mybir.AluOpType.add,
                )

            if not kt_local:
                nc.gpsimd.dma_start(
                    kt_in.rearrange("(jc p) b -> p jc b", p=128), kt_bf[:]
                )
                nc.gpsimd.collective_compute(
                    "AllGather",
                    mybir.AluOpType.bypass,
                    replica_groups=[list(range(NCORES))],
                    ins=[kt_in[:].opt()],
                    outs=[kt_all[:].opt()],
                )

            # ---- bulk prefetches (lower priority than the kT chain) ----
            if npc > 0:
                qwf_sb = sb.tile([128, npc, 8, D], BF)
                nc.sync.dma_start(
                    qwf_sb[:], qwf.rearrange("i (qc p) d -> p i qc d", p=128)
                )
            dahp_sb = sb.tile([128, n_active, D], BF)
            nc.sync.dma_start(
                dahp_sb[:], dahp.rearrange("u p d -> p u d")
            )
            zero_sb = sb.tile([128, 8], F32)
            nc.sync.dma_start(zero_sb[:], zeros4k[:])

            s_all = sb.tile([128, n_active], F32)

            def score_tile(m_ps, slot):
                """dot(dah, m)/32 -> sigmoid, one (unit, btile) score column."""
                dah_t = sb.tile(
                    [128, D], BF, name=f"dahs_t{slot}", tag="dahs_t", bufs=4
                )
                nc.sync.dma_start(dah_t[:], dahs[slot])
                mul_o = sb.tile([128, D], BF, name=f"mul_o{slot}", tag="mul_o", bufs=2)
                s_raw = sb.tile([128, 1], F32, name=f"s_raw{slot}", tag="s_raw", bufs=2)
                nc.vector.tensor_tensor(
                    mul_o[:], dah_t[:], m_ps[:], op=mybir.AluOpType.mult
                )
                nc.vector.tensor_reduce(
                    s_raw[:],
                    mul_o[:],
                    axis=mybir.AxisListType.X,
                    op=mybir.AluOpType.add,
                )
                nc.scalar.activation(
                    s_all[:, slot : slot + 1],
                    s_raw[:],
                    mybir.ActivationFunctionType.Sigmoid,
                    scale=1.0 / 32.0,
                )

            # ---- remainder units (own batch chunk, local own-kT) ----
            for r in range(nrem):
                m_ps = ps.tile([128, D], F32, name=f"mrem{r}", tag="acc", bufs=4)
                for qdc in range(8):
                    for nb in range(2):
                        nc.tensor.matmul(
                            m_ps[:, nb * 512 : (nb + 1) * 512],
                            kt_bf[:, qdc, :],
                            qwr_sb[r][:, qdc, nb * 512 : (nb + 1) * 512],
                            start=(qdc == 0),
                            stop=(qdc == 7),
                        )
                score_tile(m_ps, 8 * npc + r)

            # ---- full kT: local compute or AllGather readback ----
            # layout [p=qd%128, qdc, bt, b]
            kt_sb = sb.tile([128, 8, NCORES, PB], BF)
            if kt_local:
                for jc in range(8):
                    ktf_ps = ps.tile(
                        [128, B], F32, name=f"ktf_ps{jc}", tag="acc", bufs=4
                    )
                    for dc in range(8):
                        for nb in range(2):
                            nc.tensor.matmul(
                                ktf_ps[:, nb * 512 : (nb + 1) * 512],
                                kw_sb[dc][:, jc * 128 : (jc + 1) * 128],
                                ihf_sb[dc][:, nb * 512 : (nb + 1) * 512],
                                start=(dc == 0),
                                stop=(dc == 7),
                            )
                    nc.vector.tensor_scalar(
                        kt_sb[:, jc, :, :].rearrange("p bt b -> p (bt b)"),
                        ktf_ps[:],
                        bias_sb[:, jc : jc + 1],
                        None,
                        op0=mybir.AluOpType.add,
                    )
            else:
                nc.gpsimd.dma_start(
                    kt_sb[:].rearrange("p qc bt b -> p bt qc b"),
                    kt_all.rearrange("bt (qc p) b -> p bt qc b", p=128),
                )

            # ---- main loop: full units x batch tiles ----
            half_pt = NCORES // 2 - 1  # a2a half boundary (bt index)
            for bt in range(NCORES):
                m_f = [
                    ps.tile([128, D], F32, name=f"mf_{bt}_{i}", tag="acc", bufs=4)
                    for i in range(npc)
                ]
                for qdc in range(8):
                    for i in range(npc):
                        for nb in range(2):
                            nc.tensor.matmul(
                                m_f[i][:, nb * 512 : (nb + 1) * 512],
                                kt_sb[:, qdc, bt, :],
                                qwf_sb[:, i, qdc, nb * 512 : (nb + 1) * 512],
                                start=(qdc == 0),
                                stop=(qdc == 7),
                            )
                for i in range(npc):
                    slot = i * 8 + bt
                    score_tile(m_f[i], slot)
                    h = int(bt > half_pt) if split_a2a else 0
                    nc.gpsimd.dma_start(
                        a2a_in[h][(bt * npc + i) * 128 : (bt * npc + i + 1) * 128],
                        s_all[:, slot : slot + 1],
                    )
                if split_a2a and bt == half_pt:
                    # zero-fill the unused upper shards, then fire A2A #1
                    for j in range(half_pt + 1, NCORES):
                        nc.sync.dma_start(
                            a2a_in[0][
                                j * npc * 128 : (j + 1) * npc * 128
                            ].rearrange("(e p) -> p e", p=128),
                            zero_sb[:, :npc],
                        )
                    nc.gpsimd.collective_compute(
                        "AllToAll",
                        mybir.AluOpType.bypass,
                        replica_groups=[list(range(NCORES))],
                        ins=[a2a_in[0][:].opt()],
                        outs=[a2a_out[0][:].opt()],
                    )
            if split_a2a:
                for j in range(0, half_pt + 1):
                    nc.sync.dma_start(
                        a2a_in[1][
                            j * npc * 128 : (j + 1) * npc * 128
                        ].rearrange("(e p) -> p e", p=128),
                        zero_sb[:, :npc],
                    )

            # ---- score exchange + assembly ----
            sc_sb = sb.tile([128, n_active], F32)
            if npc > 0:
                nc.gpsimd.collective_compute(
                    "AllToAll",
                    mybir.AluOpType.bypass,
                    replica_groups=[list(range(NCORES))],
                    ins=[a2a_in[-1][:].opt()],
                    outs=[a2a_out[-1][:].opt()],
                )
                if split_a2a:
                    h1 = sb.tile([128, 8 * npc], F32)
                    nc.gpsimd.dma_start(
                        h1[:], a2a_out[0].rearrange("(ue p) -> p ue", p=128)
                    )
                    h2 = sb.tile([128, 8 * npc], F32)
                    nc.gpsimd.dma_start(
                        h2[:], a2a_out[1].rearrange("(ue p) -> p ue", p=128)
                    )
                    nc.vector.tensor_tensor(
                        sc_sb[:, 0 : 8 * npc], h1[:], h2[:], op=mybir.AluOpType.add
                    )
                else:
                    nc.gpsimd.dma_start(
                        sc_sb[:, 0 : 8 * npc],
                        a2a_out[0].rearrange("(ue p) -> p ue", p=128),
                    )
            if nrem > 0:
                nc.vector.tensor_copy(sc_sb[:, 8 * npc :], s_all[:, 8 * npc :])

            ssum = sb.tile([128, 1], F32)
            nc.vector.tensor_reduce(
                ssum[:], sc_sb[:], axis=mybir.AxisListType.X, op=mybir.AluOpType.add
            )
            rcp = sb.tile([128, 1], F32)
            nc.vector.reciprocal(rcp[:], ssum[:])

            # ---- pooling: psum = sum_u diag(s_u) @ dah_u; out = psum/ssum --
            pool_ps = ps.tile([128, D], F32, tag="acc", bufs=4)
            for u in range(n_active):
                dw = sb.tile([128, 128], BF, name=f"dw{u}", tag="dw", bufs=2)
                nc.vector.tensor_scalar(
                    dw[:],
                    ident_sb[:],
                    sc_sb[:, u : u + 1],
                    None,
                    op0=mybir.AluOpType.mult,
                )
                for nb in range(2):
                    nc.tensor.matmul(
                        pool_ps[:, nb * 512 : (nb + 1) * 512],
                        dw[:],
                        dahp_sb[:, u, nb * 512 : (nb + 1) * 512],
                        start=(u == 0),
                        stop=(u == n_active - 1),
                    )

            out_sb = sb.tile([128, D], F32)
            nc.vector.tensor_scalar(
                out_sb[:], pool_ps[:], rcp[:], None, op0=mybir.AluOpType.mult
            )
            nc.sync.dma_start(out_p[:], out_sb[:])
            nc.sync.dma_start(scores_p[:], sc_sb[:])

    nc.compile()
    return nc


def _prep_inputs(dah, ih, qw, kw, kb, n_active):
    """Host-side shard + layout prep. Returns in_maps for 8 cores."""
    kt_local = os.environ.get("BASS_KT_LOCAL", "1") == "1"
    npc = n_active // NCORES
    nrem = n_active % NCORES

    ihT = np.ascontiguousarray(ih.T).astype(BF16)                   # [D, B]
    kwT = np.ascontiguousarray(kw.T).astype(BF16)                   # [D, D]
    biasr = np.ascontiguousarray(kb.reshape(8, 128).T).astype(np.float32)
    identity = np.eye(128, dtype=BF16)
    zeros4k = np.zeros((128, 8), dtype=np.float32)
    # qwT[u] = query_weight[u].T  -> [qd, d]
    qwT = np.ascontiguousarray(
        np.transpose(qw[:n_active], (0, 2, 1))
    ).astype(BF16)
    dah_bf = dah[:, :n_active, :].astype(BF16)                      # [B, na, D]

    in_maps = []
    for c in range(NCORES):
        m = {
            "ihT": np.ascontiguousarray(ihT[:, c * PB : (c + 1) * PB]),
            "kwT": kwT,
            "biasr": biasr,
            "ident": identity,
            "zeros4k": zeros4k,
        }
        if kt_local:
            m["ihTf"] = ihT
        if npc > 0:
            m["qwf"] = np.ascontiguousarray(qwT[npc * c : npc * (c + 1)])
        for r in range(nrem):
            m[f"qwr{r}"] = qwT[8 * npc + r]
        # score pack
        slots = np.empty((n_active, PB, D), dtype=BF16)
        for i in range(npc):
            u = npc * c + i
            for bt in range(NCORES):
                slots[i * 8 + bt] = dah_bf[bt * PB : (bt + 1) * PB, u, :]
        for r in range(nrem):
            slots[8 * npc + r] = dah_bf[c * PB : (c + 1) * PB, 8 * npc + r, :]
        m["dahs"] = slots
        # pool pack
        m["dahp"] = np.ascontiguousarray(
            np.transpose(dah_bf[c * PB : (c + 1) * PB], (1, 0, 2))
        )
        in_maps.append(m)
    return in_maps


def kernel(domain_aware_h, input_h, t, query_weight, key_weight, key_bias):
    dah = np.asarray(domain_aware_h, dtype=np.float32)
    ih = np.asarray(input_h, dtype=np.float32)
    qw = np.asarray(query_weight, dtype=np.float32)
    kw = np.asarray(key_weight, dtype=np.float32)
    kb = np.asarray(key_bias, dtype=np.float32)
    t_val = int(np.asarray(t))
    n_active = max(1, min(t_val + 1, U))

    if n_active not in _BUILD_CACHE:
        _BUILD_CACHE[n_active] = _build(n_active)
    nc = _BUILD_CACHE[n_active]

    in_maps = _prep_inputs(dah, ih, qw, kw, kb, n_active)
    res = run_bass_kernel_spmd(nc, in_maps, list(range(NCORES))).results

    out = np.concatenate([res[c]["out"] for c in range(NCORES)], axis=0)
    scores = np.zeros((B, U), dtype=np.float32)
    for c in range(NCORES):
        scores[c * PB : (c + 1) * PB, :n_active] = res[c]["scores"]
    return out, scores


# revision 16
# speedup vs baseline: 1.8257x; 1.8257x over previous
"""Trainium2 Bass kernel for sparse-attention grouped-query pooling.

Reference computation (B=1024, U=32, D=1024):
    k = input_h @ key_weight.T + key_bias                  # [B, D]
    q = einsum('bud,udq->buq', domain_aware_h, query_weight)
    scores = sigmoid(einsum('buq,bq->bu', q, k) / sqrt(D)) * (arange(U) <= t)
    w = scores / scores.sum(-1, keepdims=True)
    out = (domain_aware_h * w[..., None]).sum(1)           # [B, D]
    returns (out, scores)

Only units 0..t contribute to either output (masked scores are exactly
zero), so the kernel computes just the n_active = t+1 active units.

Distribution over 8 NeuronCores (tensor-parallel by unit, with the
remainder unit split by batch):
  - npc = n_active // 8 full units per core (units npc*c + i)
  - nrem = n_active % 8 remainder units, each split across cores by
    batch chunk (core c handles batch rows [c*128, (c+1)*128))
  - score dot uses the re-association  score[b,u] = dah[b,u,:] . m_u[b,:]
    with m_u = k @ qw[u].T, so no on-device transposes are needed
    (weights are pre-transposed and pre-cast to bf16 on the host).
  - kT ([qd, b] layout of k) is computed fully on every core (34us of
    TensorE, cheaper than an AllGather which measured ~60us wall);
    per-unit scores are exchanged with two small AllToAlls (split by
    batch-tile half so the first one hides under the main loop), then
    each core does the pooling for its own batch chunk.
  - pooling runs on the TensorEngine as an accumulated diag(s_u) @ dah
    matmul with a final 1/sum(s) normalization, so the tail is short.
"""

import os
import sys

if "/opt/trn_rl_repo" not in sys.path:
    sys.path.insert(0, "/opt/trn_rl_repo")

import numpy as np
import ml_dtypes

import concourse.bass as bass
import concourse.mybir as mybir
import concourse.tile as tile
from concourse import bacc
from concourse.bass_utils import run_bass_kernel_spmd

BF16 = ml_dtypes.bfloat16
B, U, D = 1024, 32, 1024
NCORES = 8
PB = B // NCORES  # batch rows per core = 128
F32 = mybir.dt.float32
BF = mybir.dt.bfloat16

_BUILD_CACHE: dict[int, object] = {}


def _build_dp(n_active: int):
    """Zero-collective build: every core computes all units' scores for
    its OWN batch chunk (kT only needed for the local 128 batch columns),
    then pools locally. DMA-heavy (full qwT stream per core) but no
    cross-core synchronization at all."""
    nc = bacc.Bacc(None, target_bir_lowering=False, debug=False)

    ihT = nc.declare_dram_parameter("ihT", [D, PB], BF, isOutput=False)
    kwT = nc.declare_dram_parameter("kwT", [D, D], BF, isOutput=False)
    biasr = nc.declare_dram_parameter("biasr", [128, 8], F32, isOutput=False)
    ident = nc.declare_dram_parameter("ident", [128, 128], BF, isOutput=False)
    # qwa swizzled: [u][p][qc][d] so each partition reads one contiguous
    # 16 KiB run per unit; dahp swizzled: [p][u][d] (34 KB contiguous/part)
    qwa = nc.declare_dram_parameter("qwa", [n_active, 128, 8, D], BF, isOutput=False)
    dahp = nc.declare_dram_parameter("dahp", [128, n_active, D], BF, isOutput=False)

    out_p = nc.declare_dram_parameter("out", [PB, D], F32, isOutput=True)
    scores_p = nc.declare_dram_parameter("scores", [PB, n_active], F32, isOutput=True)

    GS = 2  # units per PSUM group (2 tiles + double buffering = 8 banks)

    with tile.TileContext(nc) as tc:
        with (
            tc.tile_pool(name="sb", bufs=1) as sb,
            tc.tile_pool(name="ps", bufs=1, space="PSUM") as ps,
        ):
            # ---- priority loads for the kT phase, chunked by dc ----
            kw_sb = []
            kwT_r = kwT.rearrange("(dc p) j -> dc p j", p=128)
            for dc in range(8):
                t_ = sb.tile([128, D], BF, name=f"kw_sb{dc}", tag=f"kw{dc}")
                nc.sync.dma_start(t_[:], kwT_r[dc])
                kw_sb.append(t_)
            ih_sb = sb.tile([128, 8, PB], BF)
            nc.sync.dma_start(ih_sb[:], ihT.rearrange("(dc p) b -> p dc b", p=128))
            bias_sb = sb.tile([128, 8], F32)
            nc.sync.dma_start(bias_sb[:], biasr[:])
            ident_sb = sb.tile([128, 128], BF)
            nc.sync.dma_start(ident_sb[:], ident[:])

            # ---- kT for the own batch chunk: [p=qd%128, qdc, b] ----
            kt_ps = ps.tile([128, 8, PB], F32, tag="acc", bufs=4)
            for jc in range(8):
                for dc in range(8):
                    nc.tensor.matmul(
                        kt_ps[:, jc, :],
                        kw_sb[dc][:, jc * 128 : (jc + 1) * 128],
                        ih_sb[:, dc, :],
                        start=(dc == 0),
                        stop=(dc == 7),
                    )
            kt_bf = sb.tile([128, 8, PB], BF)
            for jc in range(8):
                nc.vector.tensor_scalar(
                    kt_bf[:, jc, :],
                    kt_ps[:, jc, :],
                    bias_sb[:, jc : jc + 1],
                    None,
                    op0=mybir.AluOpType.add,
                )

            # ---- dah tiles (used by both score dot and pooling) ----
            dahp_sb = sb.tile([128, n_active, D], BF)
            nc.sync.dma_start(dahp_sb[:], dahp[:])

            # ---- scores for all units, own batch chunk ----
            sc_sb = sb.tile([128, n_active], F32)
            groups = [
                list(range(g, min(g + GS, n_active)))
                for g in range(0, n_active, GS)
            ]
            for grp in groups:
                m_g = {}
                qw_g = {}
                for u in grp:
                    qw_g[u] = sb.tile(
                        [128, 8, D], BF, name=f"qw_t{u}", tag="qw_t", bufs=3
                    )
                    # two half-loads so matmuls start on the first half
                    nc.sync.dma_start(qw_g[u][:, 0:4, :], qwa[u, :, 0:4, :])
                    nc.sync.dma_start(qw_g[u][:, 4:8, :], qwa[u, :, 4:8, :])
                    m_g[u] = ps.tile(
                        [128, D], F32, name=f"m_{u}", tag="acc", bufs=4
                    )
                for qdc in range(8):
                    for u in grp:
                        for nb in range(2):
                            nc.tensor.matmul(
                                m_g[u][:, nb * 512 : (nb + 1) * 512],
                                kt_bf[:, qdc, :],
                                qw_g[u][:, qdc, nb * 512 : (nb + 1) * 512],
                                start=(qdc == 0),
                                stop=(qdc == 7),
                            )
                for u in grp:
                    mul_o = sb.tile(
                        [128, D], BF, name=f"mul_o{u}", tag="mul_o", bufs=2
                    )
                    s_raw = sb.tile(
                        [128, 1], F32, name=f"s_raw{u}", tag="s_raw", bufs=2
                    )
                    nc.vector.tensor_tensor(
                        mul_o[:], dahp_sb[:, u, :], m_g[u][:],
                        op=mybir.AluOpType.mult,
                    )
                    nc.vector.tensor_reduce(
                        s_raw[:],
                        mul_o[:],
                        axis=mybir.AxisListType.X,
                        op=mybir.AluOpType.add,
                    )
                    nc.scalar.activation(
                        sc_sb[:, u : u + 1],
                        s_raw[:],
                        mybir.ActivationFunctionType.Sigmoid,
                        scale=1.0 / 32.0,
                    )

            ssum = sb.tile([128, 1], F32)
            nc.vector.tensor_reduce(
                ssum[:], sc_sb[:], axis=mybir.AxisListType.X, op=mybir.AluOpType.add
            )
            rcp = sb.tile([128, 1], F32)
            nc.vector.reciprocal(rcp[:], ssum[:])

            # ---- pooling: psum = sum_u diag(s_u) @ dah_u; out = psum/ssum --
            pool_ps = ps.tile([128, D], F32, tag="acc", bufs=4)
            for u in range(n_active):
                dw = sb.tile([128, 128], BF, name=f"dw{u}", tag="dw", bufs=2)
                nc.vector.tensor_scalar(
                    dw[:],
                    ident_sb[:],
                    sc_sb[:, u : u + 1],
                    None,
                    op0=mybir.AluOpType.mult,
                )
                for nb in range(2):
                    nc.tensor.matmul(
                        pool_ps[:, nb * 512 : (nb + 1) * 512],
                        dw[:],
                        dahp_sb[:, u, nb * 512 : (nb + 1) * 512],
                        start=(u == 0),
                        stop=(u == n_active - 1),
                    )

            out_sb = sb.tile([128, D], F32)
            nc.vector.tensor_scalar(
                out_sb[:], pool_ps[:], rcp[:], None, op0=mybir.AluOpType.mult
            )
            nc.sync.dma_start(out_p[:], out_sb[:])
            nc.sync.dma_start(scores_p[:], sc_sb[:])

    nc.compile()
    return nc


def _prep_inputs_dp(dah, ih, qw, kw, kb, n_active):
    ihT = np.ascontiguousarray(ih.T).astype(BF16)
    kwT = np.ascontiguousarray(kw.T).astype(BF16)
    biasr = np.ascontiguousarray(kb.reshape(8, 128).T).astype(np.float32)
    identity = np.eye(128, dtype=BF16)
    # qwT[u] = qw[u].T -> [qd, d]; swizzle to [u, p, qc, d] where
    # qd = qc*128 + p, so each partition's data is contiguous
    qwa = np.ascontiguousarray(
        np.transpose(
            np.transpose(qw[:n_active], (0, 2, 1)).reshape(n_active, 8, 128, D),
            (0, 2, 1, 3),
        )
    ).astype(BF16)
    dah_bf = dah[:, :n_active, :].astype(BF16)

    in_maps = []
    for c in range(NCORES):
        m = {
            "ihT": np.ascontiguousarray(ihT[:, c * PB : (c + 1) * PB]),
            "kwT": kwT,
            "biasr": biasr,
            "ident": identity,
            "qwa": qwa,
            "dahp": np.ascontiguousarray(dah_bf[c * PB : (c + 1) * PB]),
        }
        in_maps.append(m)
    return in_maps


def _build(n_active: int):
    """Build + compile the SPMD graph for n_active active units."""
    kt_local = os.environ.get("BASS_KT_LOCAL", "1") == "1"
    split_a2a = os.environ.get("BASS_SPLIT_A2A", "1") == "1"
    npc = n_active // NCORES        # full units per core
    nrem = n_active % NCORES        # remainder units (batch-split)

    nc = bacc.Bacc(None, target_bir_lowering=False, debug=False)

    # ---- parameters (per-core data, same graph on all cores) ----
    ihT = nc.declare_dram_parameter("ihT", [D, PB], BF, isOutput=False)
    if kt_local:
        ihTf = nc.declare_dram_parameter("ihTf", [D, B], BF, isOutput=False)
    kwT = nc.declare_dram_parameter("kwT", [D, D], BF, isOutput=False)
    biasr = nc.declare_dram_parameter("biasr", [128, 8], F32, isOutput=False)
    ident = nc.declare_dram_parameter("ident", [128, 128], BF, isOutput=False)
    zeros4k = nc.declare_dram_parameter("zeros4k", [128, 8], F32, isOutput=False)
    if npc > 0:
        qwf = nc.declare_dram_parameter("qwf", [npc, D, D], BF, isOutput=False)
    qwr = [
        nc.declare_dram_parameter(f"qwr{r}", [D, D], BF, isOutput=False)
        for r in range(nrem)
    ]
    dahs = nc.declare_dram_parameter("dahs", [n_active, PB, D], BF, isOutput=False)
    dahp = nc.declare_dram_parameter("dahp", [n_active, PB, D], BF, isOutput=False)

    out_p = nc.declare_dram_parameter("out", [PB, D], F32, isOutput=True)
    scores_p = nc.declare_dram_parameter("scores", [PB, n_active], F32, isOutput=True)

    with tile.TileContext(nc) as tc:
        with (
            tc.tile_pool(name="sb", bufs=1) as sb,
            tc.tile_pool(name="ps", bufs=1, space="PSUM") as ps,
            tc.tile_pool(name="dram", bufs=1, space="DRAM") as dram,
        ):
            # ---- collective bounce buffers ----
            if not kt_local:
                kt_in = dram.tile([D, PB], BF)
                kt_all = dram.tile([NCORES, D, PB], BF, addr_space="Shared")
            n_a2a = 2 if split_a2a else 1
            if npc > 0:
                a2a_in = [
                    dram.tile([NCORES * npc * PB], F32, name=f"a2a_in{h}")
                    for h in range(n_a2a)
                ]
                a2a_out = [
                    dram.tile([NCORES * npc * PB], F32, name=f"a2a_out{h}")
                    for h in range(n_a2a)
                ]

            # ---- priority loads: what the kT phase needs, chunked by dc ----
            kw_sb = []
            kwT_r = kwT.rearrange("(dc p) j -> dc p j", p=128)
            for dc in range(8):
                t_ = sb.tile([128, D], BF, name=f"kw_sb{dc}", tag=f"kw{dc}")
                nc.sync.dma_start(t_[:], kwT_r[dc])
                kw_sb.append(t_)
            ih_sb = sb.tile([128, 8, PB], BF)
            nc.sync.dma_start(ih_sb[:], ihT.rearrange("(dc p) b -> p dc b", p=128))
            if kt_local:
                ihf_sb = []
                ihTf_r = ihTf.rearrange("(dc p) b -> dc p b", p=128)
                for dc in range(8):
                    t_ = sb.tile([128, B], BF, name=f"ihf_sb{dc}", tag=f"ihf{dc}")
                    nc.sync.dma_start(t_[:], ihTf_r[dc])
                    ihf_sb.append(t_)
            bias_sb = sb.tile([128, 8], F32)
            nc.sync.dma_start(bias_sb[:], biasr[:])
            ident_sb = sb.tile([128, 128], BF)
            nc.sync.dma_start(ident_sb[:], ident[:])
            qwr_sb = []
            for r in range(nrem):
                t_ = sb.tile([128, 8, D], BF, name=f"qwr_sb{r}", tag=f"qwr{r}")
                nc.sync.dma_start(t_[:], qwr[r].rearrange("(qc p) d -> p qc d", p=128))
                qwr_sb.append(t_)

            # ---- kT-own phase (for the remainder units): kT[:, own chunk] ----
            kt_ps = ps.tile([128, 8, PB], F32, tag="acc", bufs=4)
            for jc in range(8):
                for dc in range(8):
                    nc.tensor.matmul(
                        kt_ps[:, jc, :],
                        kw_sb[dc][:, jc * 128 : (jc + 1) * 128],
                        ih_sb[:, dc, :],
                        start=(dc == 0),
                        stop=(dc == 7),
                    )
            kt_bf = sb.tile([128, 8, PB], BF)
            for jc in range(8):
                nc.vector.tensor_scalar(
                    kt_bf[:, jc, :],
                    kt_ps[:, jc, :],
                    bias_sb[:, jc : jc + 1],
                    None,
                    op0=mybir.AluOpType.add,
                )

            if not kt_local:
                nc.gpsimd.dma_start(
                    kt_in.rearrange("(jc p) b -> p jc b", p=128), kt_bf[:]
                )
                nc.gpsimd.collective_compute(
                    "AllGather",
                    mybir.AluOpType.bypass,
                    replica_groups=[list(range(NCORES))],
                    ins=[kt_in[:].opt()],
                    outs=[kt_all[:].opt()],
                )

            # ---- bulk prefetches (lower priority than the kT chain) ----
            if npc > 0:
                qwf_sb = sb.tile([128, npc, 8, D], BF)
                nc.sync.dma_start(
                    qwf_sb[:], qwf.rearrange("i (qc p) d -> p i qc d", p=128)
                )
            dahp_sb = sb.tile([128, n_active, D], BF)
            nc.sync.dma_start(
                dahp_sb[:], dahp.rearrange("u p d -> p u d")
            )
            zero_sb = sb.tile([128, 8], F32)
            nc.sync.dma_start(zero_sb[:], zeros4k[:])

            s_all = sb.tile([128, n_active], F32)

            def score_tile(m_ps, slot):
                """dot(dah, m)/32 -> sigmoid, one (unit, btile) score column."""
                dah_t = sb.tile(
                    [128, D], BF, name=f"dahs_t{slot}", tag="dahs_t", bufs=4
                )
                nc.sync.dma_start(dah_t[:], dahs[slot])
                mul_o = sb.tile([128, D], BF, name=f"mul_o{slot}", tag="mul_o", bufs=2)
                s_raw = sb.tile([128, 1], F32, name=f"s_raw{slot}", tag="s_raw", bufs=2)
                nc.vector.tensor_tensor(
                    mul_o[:], dah_t[:], m_ps[:], op=mybir.AluOpType.mult
                )
                nc.vector.tensor_reduce(
                    s_raw[:],
                    mul_o[:],
                    axis=mybir.AxisListType.X,
                    op=mybir.AluOpType.add,
                )
                nc.scalar.activation(
                    s_all[:, slot : slot + 1],
                    s_raw[:],
                    mybir.ActivationFunctionType.Sigmoid,
                    scale=1.0 / 32.0,
                )

            # ---- remainder units (own batch chunk, local own-kT) ----
            for r in range(nrem):
                m_ps = ps.tile([128, D], F32, name=f"mrem{r}", tag="acc", bufs=4)
                for qdc in range(8):
                    for nb in range(2):
                        nc.tensor.matmul(
                            m_ps[:, nb * 512 : (nb + 1) * 512],
                            kt_bf[:, qdc, :],
                            qwr_sb[r][:, qdc, nb * 512 : (nb + 1) * 512],
                            start=(qdc == 0),
                            stop=(qdc == 7),
                        )
                score_tile(m_ps, 8 * npc + r)

            # ---- full kT: local compute or AllGather readback ----
            # layout [p=qd%128, qdc, bt, b]
            kt_sb = sb.tile([128, 8, NCORES, PB], BF)
            if kt_local:
                for jc in range(8):
                    ktf_ps = ps.tile(
                        [128, B], F32, name=f"ktf_ps{jc}", tag="acc", bufs=4
                    )
                    for dc in range(8):
                        for nb in range(2):
                            nc.tensor.matmul(
                                ktf_ps[:, nb * 512 : (nb + 1) * 512],
                                kw_sb[dc][:, jc * 128 : (jc + 1) * 128],
                                ihf_sb[dc][:, nb * 512 : (nb + 1) * 512],
                                start=(dc == 0),
                                stop=(dc == 7),
                            )
                    nc.vector.tensor_scalar(
                        kt_sb[:, jc, :, :].rearrange("p bt b -> p (bt b)"),
                        ktf_ps[:],
                        bias_sb[:, jc : jc + 1],
                        None,
                        op0=mybir.AluOpType.add,
                    )
            else:
                nc.gpsimd.dma_start(
                    kt_sb[:].rearrange("p qc bt b -> p bt qc b"),
                    kt_all.rearrange("bt (qc p) b -> p bt qc b", p=128),
                )

            # ---- main loop: full units x batch tiles ----
            half_pt = NCORES // 2 - 1  # a2a half boundary (bt index)
            for bt in range(NCORES):
                m_f = [
                    ps.tile([128, D], F32, name=f"mf_{bt}_{i}", tag="acc", bufs=4)
                    for i in range(npc)
                ]
                for qdc in range(8):
                    for i in range(npc):
                        for nb in range(2):
                            nc.tensor.matmul(
                                m_f[i][:, nb * 512 : (nb + 1) * 512],
                                kt_sb[:, qdc, bt, :],
                                qwf_sb[:, i, qdc, nb * 512 : (nb + 1) * 512],
                                start=(qdc == 0),
                                stop=(qdc == 7),
                            )
                for i in range(npc):
                    slot = i * 8 + bt
                    score_tile(m_f[i], slot)
                    h = int(bt > half_pt) if split_a2a else 0
                    nc.gpsimd.dma_start(
                        a2a_in[h][(bt * npc + i) * 128 : (bt * npc + i + 1) * 128],
                        s_all[:, slot : slot + 1],
                    )
                if split_a2a and bt == half_pt:
                    # zero-fill the unused upper shards, then fire A2A #1
                    for j in range(half_pt + 1, NCORES):
                        nc.sync.dma_start(
                            a2a_in[0][
                                j * npc * 128 : (j + 1) * npc * 128
                            ].rearrange("(e p) -> p e", p=128),
                            zero_sb[:, :npc],
                        )
                    nc.gpsimd.collective_compute(
                        "AllToAll",
                        mybir.AluOpType.bypass,
                        replica_groups=[list(range(NCORES))],
                        ins=[a2a_in[0][:].opt()],
                        outs=[a2a_out[0][:].opt()],
                    )
            if split_a2a:
                for j in range(0, half_pt + 1):
                    nc.sync.dma_start(
                        a2a_in[1][
                            j * npc * 128 : (j + 1) * npc * 128
                        ].rearrange("(e p) -> p e", p=128),
                        zero_sb[:, :npc],
                    )

            # ---- score exchange + assembly ----
            sc_sb = sb.tile([128, n_active], F32)
            if npc > 0:
                nc.gpsimd.collective_compute(
                    "AllToAll",
                    mybir.AluOpType.bypass,
                    replica_groups=[list(range(NCORES))],
                    ins=[a2a_in[-1][:].opt()],
                    outs=[a2a_out[-1][:].opt()],
                )
                if split_a2a:
                    h1 = sb.tile([128, 8 * npc], F32)
                    nc.gpsimd.dma_start(
                        h1[:], a2a_out[0].rearrange("(ue p) -> p ue", p=128)
                    )
                    h2 = sb.tile([128, 8 * npc], F32)
                    nc.gpsimd.dma_start(
                        h2[:], a2a_out[1].rearrange("(ue p) -> p ue", p=128)
                    )
                    nc.vector.tensor_tensor(
                        sc_sb[:, 0 : 8 * npc], h1[:], h2[:], op=mybir.AluOpType.add
                    )
                else:
                    nc.gpsimd.dma_start(
                        sc_sb[:, 0 : 8 * npc],
                        a2a_out[0].rearrange("(ue p) -> p ue", p=128),
                    )
            if nrem > 0:
                nc.vector.tensor_copy(sc_sb[:, 8 * npc :], s_all[:, 8 * npc :])

            ssum = sb.tile([128, 1], F32)
            nc.vector.tensor_reduce(
                ssum[:], sc_sb[:], axis=mybir.AxisListType.X, op=mybir.AluOpType.add
            )
            rcp = sb.tile([128, 1], F32)
            nc.vector.reciprocal(rcp[:], ssum[:])

            # ---- pooling: psum = sum_u diag(s_u) @ dah_u; out = psum/ssum --
            pool_ps = ps.tile([128, D], F32, tag="acc", bufs=4)
            for u in range(n_active):
                dw = sb.tile([128, 128], BF, name=f"dw{u}", tag="dw", bufs=2)
                nc.vector.tensor_scalar(
                    dw[:],
                    ident_sb[:],
                    sc_sb[:, u : u + 1],
                    None,
                    op0=mybir.AluOpType.mult,
                )
                for nb in range(2):
                    nc.tensor.matmul(
                        pool_ps[:, nb * 512 : (nb + 1) * 512],
                        dw[:],
                        dahp_sb[:, u, nb * 512 : (nb + 1) * 512],
                        start=(u == 0),
                        stop=(u == n_active - 1),
                    )

            out_sb = sb.tile([128, D], F32)
            nc.vector.tensor_scalar(
                out_sb[:], pool_ps[:], rcp[:], None, op0=mybir.AluOpType.mult
            )
            nc.sync.dma_start(out_p[:], out_sb[:])
            nc.sync.dma_start(scores_p[:], sc_sb[:])

    nc.compile()
    return nc


def _prep_inputs(dah, ih, qw, kw, kb, n_active):
    """Host-side shard + layout prep. Returns in_maps for 8 cores."""
    kt_local = os.environ.get("BASS_KT_LOCAL", "1") == "1"
    npc = n_active // NCORES
    nrem = n_active % NCORES

    ihT = np.ascontiguousarray(ih.T).astype(BF16)                   # [D, B]
    kwT = np.ascontiguousarray(kw.T).astype(BF16)                   # [D, D]
    biasr = np.ascontiguousarray(kb.reshape(8, 128).T).astype(np.float32)
    identity = np.eye(128, dtype=BF16)
    zeros4k = np.zeros((128, 8), dtype=np.float32)
    # qwT[u] = query_weight[u].T  -> [qd, d]
    qwT = np.ascontiguousarray(
        np.transpose(qw[:n_active], (0, 2, 1))
    ).astype(BF16)
    dah_bf = dah[:, :n_active, :].astype(BF16)                      # [B, na, D]

    in_maps = []
    for c in range(NCORES):
        m = {
            "ihT": np.ascontiguousarray(ihT[:, c * PB : (c + 1) * PB]),
            "kwT": kwT,
            "biasr": biasr,
            "ident": identity,
            "zeros4k": zeros4k,
        }
        if kt_local:
            m["ihTf"] = ihT
        if npc > 0:
            m["qwf"] = np.ascontiguousarray(qwT[npc * c : npc * (c + 1)])
        for r in range(nrem):
            m[f"qwr{r}"] = qwT[8 * npc + r]
        # score pack
        slots = np.empty((n_active, PB, D), dtype=BF16)
        for i in range(npc):
            u = npc * c + i
            for bt in range(NCORES):
                slots[i * 8 + bt] = dah_bf[bt * PB : (bt + 1) * PB, u, :]
        for r in range(nrem):
            slots[8 * npc + r] = dah_bf[c * PB : (c + 1) * PB, 8 * npc + r, :]
        m["dahs"] = slots
        # pool pack
        m["dahp"] = np.ascontiguousarray(
            np.transpose(dah_bf[c * PB : (c + 1) * PB], (1, 0, 2))
        )
        in_maps.append(m)
    return in_maps


def kernel(domain_aware_h, input_h, t, query_weight, key_weight, key_bias):
    dah = np.asarray(domain_aware_h, dtype=np.float32)
    ih = np.asarray(input_h, dtype=np.float32)
    qw = np.asarray(query_weight, dtype=np.float32)
    kw = np.asarray(key_weight, dtype=np.float32)
    kb = np.asarray(key_bias, dtype=np.float32)
    t_val = int(np.asarray(t))
    n_active = max(1, min(t_val + 1, U))

    mode = os.environ.get("BASS_MODE", "dp")
    key = (mode, n_active)
    if key not in _BUILD_CACHE:
        _BUILD_CACHE[key] = (
            _build_dp(n_active) if mode == "dp" else _build(n_active)
        )
    nc = _BUILD_CACHE[key]

    if mode == "dp":
        in_maps = _prep_inputs_dp(dah, ih, qw, kw, kb, n_active)
    else:
        in_maps = _prep_inputs(dah, ih, qw, kw, kb, n_active)
    res = run_bass_kernel_spmd(nc, in_maps, list(range(NCORES))).results

    out = np.concatenate([res[c]["out"] for c in range(NCORES)], axis=0)
    scores = np.zeros((B, U), dtype=np.float32)
    for c in range(NCORES):
        scores[c * PB : (c + 1) * PB, :n_active] = res[c]["scores"]
    return out, scores


# revision 17
# speedup vs baseline: 1.8326x; 1.0038x over previous
"""Trainium2 Bass kernel for sparse-attention grouped-query pooling.

Reference computation (B=1024, U=32, D=1024):
    k = input_h @ key_weight.T + key_bias                  # [B, D]
    q = einsum('bud,udq->buq', domain_aware_h, query_weight)
    scores = sigmoid(einsum('buq,bq->bu', q, k) / sqrt(D)) * (arange(U) <= t)
    w = scores / scores.sum(-1, keepdims=True)
    out = (domain_aware_h * w[..., None]).sum(1)           # [B, D]
    returns (out, scores)

Only units 0..t contribute to either output (masked scores are exactly
zero), so the kernel computes just the n_active = t+1 active units.

Distribution over 8 NeuronCores (tensor-parallel by unit, with the
remainder unit split by batch):
  - npc = n_active // 8 full units per core (units npc*c + i)
  - nrem = n_active % 8 remainder units, each split across cores by
    batch chunk (core c handles batch rows [c*128, (c+1)*128))
  - score dot uses the re-association  score[b,u] = dah[b,u,:] . m_u[b,:]
    with m_u = k @ qw[u].T, so no on-device transposes are needed
    (weights are pre-transposed and pre-cast to bf16 on the host).
  - kT ([qd, b] layout of k) is computed fully on every core (34us of
    TensorE, cheaper than an AllGather which measured ~60us wall);
    per-unit scores are exchanged with two small AllToAlls (split by
    batch-tile half so the first one hides under the main loop), then
    each core does the pooling for its own batch chunk.
  - pooling runs on the TensorEngine as an accumulated diag(s_u) @ dah
    matmul with a final 1/sum(s) normalization, so the tail is short.
"""

import os
import sys

if "/opt/trn_rl_repo" not in sys.path:
    sys.path.insert(0, "/opt/trn_rl_repo")

import numpy as np
import ml_dtypes

import concourse.bass as bass
import concourse.mybir as mybir
import concourse.tile as tile
from concourse import bacc
from concourse.bass_utils import run_bass_kernel_spmd

BF16 = ml_dtypes.bfloat16
B, U, D = 1024, 32, 1024
NCORES = 8
PB = B // NCORES  # batch rows per core = 128
F32 = mybir.dt.float32
BF = mybir.dt.bfloat16

_BUILD_CACHE: dict[int, object] = {}


def _build_dp(n_active: int):
    """Zero-collective build: every core computes all units' scores for
    its OWN batch chunk (kT only needed for the local 128 batch columns),
    then pools locally. DMA-heavy (full qwT stream per core) but no
    cross-core synchronization at all."""
    nc = bacc.Bacc(None, target_bir_lowering=False, debug=False)

    ihT = nc.declare_dram_parameter("ihT", [D, PB], BF, isOutput=False)
    kwT = nc.declare_dram_parameter("kwT", [D, D], BF, isOutput=False)
    biasr = nc.declare_dram_parameter("biasr", [128, 8], F32, isOutput=False)
    ident = nc.declare_dram_parameter("ident", [128, 128], BF, isOutput=False)
    # qwa swizzled: [u][p][qc][d] so each partition reads one contiguous
    # 16 KiB run per unit; dahp swizzled: [p][u][d] (34 KB contiguous/part)
    qwa = nc.declare_dram_parameter("qwa", [n_active, 128, 8, D], BF, isOutput=False)
    dahp = nc.declare_dram_parameter("dahp", [128, n_active, D], BF, isOutput=False)

    out_p = nc.declare_dram_parameter("out", [PB, D], F32, isOutput=True)
    scores_p = nc.declare_dram_parameter("scores", [PB, n_active], F32, isOutput=True)

    GS = 2  # units per PSUM group (2 tiles + double buffering = 8 banks)

    with tile.TileContext(nc) as tc:
        with (
            tc.tile_pool(name="sb", bufs=1) as sb,
            tc.tile_pool(name="ps", bufs=1, space="PSUM") as ps,
        ):
            # ---- priority loads for the kT phase, chunked by dc ----
            kw_sb = []
            kwT_r = kwT.rearrange("(dc p) j -> dc p j", p=128)
            for dc in range(8):
                t_ = sb.tile([128, D], BF, name=f"kw_sb{dc}", tag=f"kw{dc}")
                nc.sync.dma_start(t_[:], kwT_r[dc])
                kw_sb.append(t_)
            ih_sb = sb.tile([128, 8, PB], BF)
            nc.sync.dma_start(ih_sb[:], ihT.rearrange("(dc p) b -> p dc b", p=128))
            bias_sb = sb.tile([128, 8], F32)
            nc.sync.dma_start(bias_sb[:], biasr[:])
            ident_sb = sb.tile([128, 128], BF)
            nc.sync.dma_start(ident_sb[:], ident[:])

            # ---- kT for the own batch chunk: [p=qd%128, qdc, b] ----
            kt_ps = ps.tile([128, 8, PB], F32, tag="acc", bufs=4)
            for jc in range(8):
                for dc in range(8):
                    nc.tensor.matmul(
                        kt_ps[:, jc, :],
                        kw_sb[dc][:, jc * 128 : (jc + 1) * 128],
                        ih_sb[:, dc, :],
                        start=(dc == 0),
                        stop=(dc == 7),
                    )
            kt_bf = sb.tile([128, 8, PB], BF)
            for jc in range(8):
                nc.vector.tensor_scalar(
                    kt_bf[:, jc, :],
                    kt_ps[:, jc, :],
                    bias_sb[:, jc : jc + 1],
                    None,
                    op0=mybir.AluOpType.add,
                )

            # ---- dah tiles (used by both score dot and pooling) ----
            dahp_sb = sb.tile([128, n_active, D], BF)

            # ---- scores for all units, own batch chunk ----
            sc_sb = sb.tile([128, n_active], F32)
            groups = [
                list(range(g, min(g + GS, n_active)))
                for g in range(0, n_active, GS)
            ]
            dahp_cuts = [0, min(6, n_active), min(12, n_active), n_active]
            dahp_done = 0
            for gi, grp in enumerate(groups):
                m_g = {}
                qw_g = {}
                for u in grp:
                    qw_g[u] = sb.tile(
                        [128, 8, D], BF, name=f"qw_t{u}", tag="qw_t", bufs=5
                    )
                    # two half-loads so matmuls start on the first half
                    nc.sync.dma_start(qw_g[u][:, 0:4, :], qwa[u, :, 0:4, :])
                    nc.sync.dma_start(qw_g[u][:, 4:8, :], qwa[u, :, 4:8, :])
                    m_g[u] = ps.tile(
                        [128, D], F32, name=f"m_{u}", tag="acc", bufs=4
                    )
                # interleave dah chunk loads between qw groups
                if gi < 3 and dahp_done < len(dahp_cuts) - 1:
                    lo, hi = dahp_cuts[dahp_done], dahp_cuts[dahp_done + 1]
                    if hi > lo:
                        nc.sync.dma_start(dahp_sb[:, lo:hi, :], dahp[:, lo:hi, :])
                    dahp_done += 1
                for qdc in range(8):
                    for u in grp:
                        for nb in range(2):
                            nc.tensor.matmul(
                                m_g[u][:, nb * 512 : (nb + 1) * 512],
                                kt_bf[:, qdc, :],
                                qw_g[u][:, qdc, nb * 512 : (nb + 1) * 512],
                                start=(qdc == 0),
                                stop=(qdc == 7),
                            )
                use_stt = os.environ.get("BASS_STT", "1") == "1"
                for u in grp:
                    mul_o = sb.tile(
                        [128, D], BF, name=f"mul_o{u}", tag="mul_o", bufs=2
                    )
                    s_raw = sb.tile(
                        [128, 1], F32, name=f"s_raw{u}", tag="s_raw", bufs=2
                    )
                    if use_stt:
                        nc.vector.scalar_tensor_tensor(
                            mul_o[:],
                            dahp_sb[:, u, :],
                            1.0,
                            m_g[u][:],
                            op0=mybir.AluOpType.mult,
                            op1=mybir.AluOpType.mult,
                            accum_out=s_raw[:],
                        )
                    else:
                        nc.vector.tensor_tensor(
                            mul_o[:], dahp_sb[:, u, :], m_g[u][:],
                            op=mybir.AluOpType.mult,
                        )
                        nc.vector.tensor_reduce(
                            s_raw[:],
                            mul_o[:],
                            axis=mybir.AxisListType.X,
                            op=mybir.AluOpType.add,
                        )
                    nc.scalar.activation(
                        sc_sb[:, u : u + 1],
                        s_raw[:],
                        mybir.ActivationFunctionType.Sigmoid,
                        scale=1.0 / 32.0,
                    )

            ssum = sb.tile([128, 1], F32)
            nc.vector.tensor_reduce(
                ssum[:], sc_sb[:], axis=mybir.AxisListType.X, op=mybir.AluOpType.add
            )
            rcp = sb.tile([128, 1], F32)
            nc.vector.reciprocal(rcp[:], ssum[:])

            # ---- pooling: psum = sum_u diag(s_u) @ dah_u; out = psum/ssum --
            pool_ps = ps.tile([128, D], F32, tag="acc", bufs=4)
            for u in range(n_active):
                dw = sb.tile([128, 128], BF, name=f"dw{u}", tag="dw", bufs=2)
                nc.vector.tensor_scalar(
                    dw[:],
                    ident_sb[:],
                    sc_sb[:, u : u + 1],
                    None,
                    op0=mybir.AluOpType.mult,
                )
                for nb in range(2):
                    nc.tensor.matmul(
                        pool_ps[:, nb * 512 : (nb + 1) * 512],
                        dw[:],
                        dahp_sb[:, u, nb * 512 : (nb + 1) * 512],
                        start=(u == 0),
                        stop=(u == n_active - 1),
                    )

            out_sb = sb.tile([128, D], F32)
            nc.vector.tensor_scalar(
                out_sb[:], pool_ps[:], rcp[:], None, op0=mybir.AluOpType.mult
            )
            nc.sync.dma_start(out_p[:], out_sb[:])
            nc.sync.dma_start(scores_p[:], sc_sb[:])

    nc.compile()
    return nc


def _prep_inputs_dp(dah, ih, qw, kw, kb, n_active):
    ihT = np.ascontiguousarray(ih.T).astype(BF16)
    kwT = np.ascontiguousarray(kw.T).astype(BF16)
    biasr = np.ascontiguousarray(kb.reshape(8, 128).T).astype(np.float32)
    identity = np.eye(128, dtype=BF16)
    # qwT[u] = qw[u].T -> [qd, d]; swizzle to [u, p, qc, d] where
    # qd = qc*128 + p, so each partition's data is contiguous
    qwa = np.ascontiguousarray(
        np.transpose(
            np.transpose(qw[:n_active], (0, 2, 1)).reshape(n_active, 8, 128, D),
            (0, 2, 1, 3),
        )
    ).astype(BF16)
    dah_bf = dah[:, :n_active, :].astype(BF16)

    in_maps = []
    for c in range(NCORES):
        m = {
            "ihT": np.ascontiguousarray(ihT[:, c * PB : (c + 1) * PB]),
            "kwT": kwT,
            "biasr": biasr,
            "ident": identity,
            "qwa": qwa,
            "dahp": np.ascontiguousarray(dah_bf[c * PB : (c + 1) * PB]),
        }
        in_maps.append(m)
    return in_maps


def _build(n_active: int):
    """Build + compile the SPMD graph for n_active active units."""
    kt_local = os.environ.get("BASS_KT_LOCAL", "1") == "1"
    split_a2a = os.environ.get("BASS_SPLIT_A2A", "1") == "1"
    npc = n_active // NCORES        # full units per core
    nrem = n_active % NCORES        # remainder units (batch-split)

    nc = bacc.Bacc(None, target_bir_lowering=False, debug=False)

    # ---- parameters (per-core data, same graph on all cores) ----
    ihT = nc.declare_dram_parameter("ihT", [D, PB], BF, isOutput=False)
    if kt_local:
        ihTf = nc.declare_dram_parameter("ihTf", [D, B], BF, isOutput=False)
    kwT = nc.declare_dram_parameter("kwT", [D, D], BF, isOutput=False)
    biasr = nc.declare_dram_parameter("biasr", [128, 8], F32, isOutput=False)
    ident = nc.declare_dram_parameter("ident", [128, 128], BF, isOutput=False)
    zeros4k = nc.declare_dram_parameter("zeros4k", [128, 8], F32, isOutput=False)
    if npc > 0:
        qwf = nc.declare_dram_parameter("qwf", [npc, D, D], BF, isOutput=False)
    qwr = [
        nc.declare_dram_parameter(f"qwr{r}", [D, D], BF, isOutput=False)
        for r in range(nrem)
    ]
    dahs = nc.declare_dram_parameter("dahs", [n_active, PB, D], BF, isOutput=False)
    dahp = nc.declare_dram_parameter("dahp", [n_active, PB, D], BF, isOutput=False)

    out_p = nc.declare_dram_parameter("out", [PB, D], F32, isOutput=True)
    scores_p = nc.declare_dram_parameter("scores", [PB, n_active], F32, isOutput=True)

    with tile.TileContext(nc) as tc:
        with (
            tc.tile_pool(name="sb", bufs=1) as sb,
            tc.tile_pool(name="ps", bufs=1, space="PSUM") as ps,
            tc.tile_pool(name="dram", bufs=1, space="DRAM") as dram,
        ):
            # ---- collective bounce buffers ----
            if not kt_local:
                kt_in = dram.tile([D, PB], BF)
                kt_all = dram.tile([NCORES, D, PB], BF, addr_space="Shared")
            n_a2a = 2 if split_a2a else 1
            if npc > 0:
                a2a_in = [
                    dram.tile([NCORES * npc * PB], F32, name=f"a2a_in{h}")
                    for h in range(n_a2a)
                ]
                a2a_out = [
                    dram.tile([NCORES * npc * PB], F32, name=f"a2a_out{h}")
                    for h in range(n_a2a)
                ]

            # ---- priority loads: what the kT phase needs, chunked by dc ----
            kw_sb = []
            kwT_r = kwT.rearrange("(dc p) j -> dc p j", p=128)
            for dc in range(8):
                t_ = sb.tile([128, D], BF, name=f"kw_sb{dc}", tag=f"kw{dc}")
                nc.sync.dma_start(t_[:], kwT_r[dc])
                kw_sb.append(t_)
            ih_sb = sb.tile([128, 8, PB], BF)
            nc.sync.dma_start(ih_sb[:], ihT.rearrange("(dc p) b -> p dc b", p=128))
            if kt_local:
                ihf_sb = []
                ihTf_r = ihTf.rearrange("(dc p) b -> dc p b", p=128)
                for dc in range(8):
                    t_ = sb.tile([128, B], BF, name=f"ihf_sb{dc}", tag=f"ihf{dc}")
                    nc.sync.dma_start(t_[:], ihTf_r[dc])
                    ihf_sb.append(t_)
            bias_sb = sb.tile([128, 8], F32)
            nc.sync.dma_start(bias_sb[:], biasr[:])
            ident_sb = sb.tile([128, 128], BF)
            nc.sync.dma_start(ident_sb[:], ident[:])
            qwr_sb = []
            for r in range(nrem):
                t_ = sb.tile([128, 8, D], BF, name=f"qwr_sb{r}", tag=f"qwr{r}")
                nc.sync.dma_start(t_[:], qwr[r].rearrange("(qc p) d -> p qc d", p=128))
                qwr_sb.append(t_)

            # ---- kT-own phase (for the remainder units): kT[:, own chunk] ----
            kt_ps = ps.tile([128, 8, PB], F32, tag="acc", bufs=4)
            for jc in range(8):
                for dc in range(8):
                    nc.tensor.matmul(
                        kt_ps[:, jc, :],
                        kw_sb[dc][:, jc * 128 : (jc + 1) * 128],
                        ih_sb[:, dc, :],
                        start=(dc == 0),
                        stop=(dc == 7),
                    )
            kt_bf = sb.tile([128, 8, PB], BF)
            for jc in range(8):
                nc.vector.tensor_scalar(
                    kt_bf[:, jc, :],
                    kt_ps[:, jc, :],
                    bias_sb[:, jc : jc + 1],
                    None,
                    op0=mybir.AluOpType.add,
                )

            if not kt_local:
                nc.gpsimd.dma_start(
                    kt_in.rearrange("(jc p) b -> p jc b", p=128), kt_bf[:]
                )
                nc.gpsimd.collective_compute(
                    "AllGather",
                    mybir.AluOpType.bypass,
                    replica_groups=[list(range(NCORES))],
                    ins=[kt_in[:].opt()],
                    outs=[kt_all[:].opt()],
                )

            # ---- bulk prefetches (lower priority than the kT chain) ----
            if npc > 0:
                qwf_sb = sb.tile([128, npc, 8, D], BF)
                nc.sync.dma_start(
                    qwf_sb[:], qwf.rearrange("i (qc p) d -> p i qc d", p=128)
                )
            dahp_sb = sb.tile([128, n_active, D], BF)
            nc.sync.dma_start(
                dahp_sb[:], dahp.rearrange("u p d -> p u d")
            )
            zero_sb = sb.tile([128, 8], F32)
            nc.sync.dma_start(zero_sb[:], zeros4k[:])

            s_all = sb.tile([128, n_active], F32)

            def score_tile(m_ps, slot):
                """dot(dah, m)/32 -> sigmoid, one (unit, btile) score column."""
                dah_t = sb.tile(
                    [128, D], BF, name=f"dahs_t{slot}", tag="dahs_t", bufs=4
                )
                nc.sync.dma_start(dah_t[:], dahs[slot])
                mul_o = sb.tile([128, D], BF, name=f"mul_o{slot}", tag="mul_o", bufs=2)
                s_raw = sb.tile([128, 1], F32, name=f"s_raw{slot}", tag="s_raw", bufs=2)
                nc.vector.tensor_tensor(
                    mul_o[:], dah_t[:], m_ps[:], op=mybir.AluOpType.mult
                )
                nc.vector.tensor_reduce(
                    s_raw[:],
                    mul_o[:],
                    axis=mybir.AxisListType.X,
                    op=mybir.AluOpType.add,
                )
                nc.scalar.activation(
                    s_all[:, slot : slot + 1],
                    s_raw[:],
                    mybir.ActivationFunctionType.Sigmoid,
                    scale=1.0 / 32.0,
                )

            # ---- remainder units (own batch chunk, local own-kT) ----
            for r in range(nrem):
                m_ps = ps.tile([128, D], F32, name=f"mrem{r}", tag="acc", bufs=4)
                for qdc in range(8):
                    for nb in range(2):
                        nc.tensor.matmul(
                            m_ps[:, nb * 512 : (nb + 1) * 512],
                            kt_bf[:, qdc, :],
                            qwr_sb[r][:, qdc, nb * 512 : (nb + 1) * 512],
                            start=(qdc == 0),
                            stop=(qdc == 7),
                        )
                score_tile(m_ps, 8 * npc + r)

            # ---- full kT: local compute or AllGather readback ----
            # layout [p=qd%128, qdc, bt, b]
            kt_sb = sb.tile([128, 8, NCORES, PB], BF)
            if kt_local:
                for jc in range(8):
                    ktf_ps = ps.tile(
                        [128, B], F32, name=f"ktf_ps{jc}", tag="acc", bufs=4
                    )
                    for dc in range(8):
                        for nb in range(2):
                            nc.tensor.matmul(
                                ktf_ps[:, nb * 512 : (nb + 1) * 512],
                                kw_sb[dc][:, jc * 128 : (jc + 1) * 128],
                                ihf_sb[dc][:, nb * 512 : (nb + 1) * 512],
                                start=(dc == 0),
                                stop=(dc == 7),
                            )
                    nc.vector.tensor_scalar(
                        kt_sb[:, jc, :, :].rearrange("p bt b -> p (bt b)"),
                        ktf_ps[:],
                        bias_sb[:, jc : jc + 1],
                        None,
                        op0=mybir.AluOpType.add,
                    )
            else:
                nc.gpsimd.dma_start(
                    kt_sb[:].rearrange("p qc bt b -> p bt qc b"),
                    kt_all.rearrange("bt (qc p) b -> p bt qc b", p=128),
                )

            # ---- main loop: full units x batch tiles ----
            half_pt = NCORES // 2 - 1  # a2a half boundary (bt index)
            for bt in range(NCORES):
                m_f = [
                    ps.tile([128, D], F32, name=f"mf_{bt}_{i}", tag="acc", bufs=4)
                    for i in range(npc)
                ]
                for qdc in range(8):
                    for i in range(npc):
                        for nb in range(2):
                            nc.tensor.matmul(
                                m_f[i][:, nb * 512 : (nb + 1) * 512],
                                kt_sb[:, qdc, bt, :],
                                qwf_sb[:, i, qdc, nb * 512 : (nb + 1) * 512],
                                start=(qdc == 0),
                                stop=(qdc == 7),
                            )
                for i in range(npc):
                    slot = i * 8 + bt
                    score_tile(m_f[i], slot)
                    h = int(bt > half_pt) if split_a2a else 0
                    nc.gpsimd.dma_start(
                        a2a_in[h][(bt * npc + i) * 128 : (bt * npc + i + 1) * 128],
                        s_all[:, slot : slot + 1],
                    )
                if split_a2a and bt == half_pt:
                    # zero-fill the unused upper shards, then fire A2A #1
                    for j in range(half_pt + 1, NCORES):
                        nc.sync.dma_start(
                            a2a_in[0][
                                j * npc * 128 : (j + 1) * npc * 128
                            ].rearrange("(e p) -> p e", p=128),
                            zero_sb[:, :npc],
                        )
                    nc.gpsimd.collective_compute(
                        "AllToAll",
                        mybir.AluOpType.bypass,
                        replica_groups=[list(range(NCORES))],
                        ins=[a2a_in[0][:].opt()],
                        outs=[a2a_out[0][:].opt()],
                    )
            if split_a2a:
                for j in range(0, half_pt + 1):
                    nc.sync.dma_start(
                        a2a_in[1][
                            j * npc * 128 : (j + 1) * npc * 128
                        ].rearrange("(e p) -> p e", p=128),
                        zero_sb[:, :npc],
                    )

            # ---- score exchange + assembly ----
            sc_sb = sb.tile([128, n_active], F32)
            if npc > 0:
                nc.gpsimd.collective_compute(
                    "AllToAll",
                    mybir.AluOpType.bypass,
                    replica_groups=[list(range(NCORES))],
                    ins=[a2a_in[-1][:].opt()],
                    outs=[a2a_out[-1][:].opt()],
                )
                if split_a2a:
                    h1 = sb.tile([128, 8 * npc], F32)
                    nc.gpsimd.dma_start(
                        h1[:], a2a_out[0].rearrange("(ue p) -> p ue", p=128)
                    )
                    h2 = sb.tile([128, 8 * npc], F32)
                    nc.gpsimd.dma_start(
                        h2[:], a2a_out[1].rearrange("(ue p) -> p ue", p=128)
                    )
                    nc.vector.tensor_tensor(
                        sc_sb[:, 0 : 8 * npc], h1[:], h2[:], op=mybir.AluOpType.add
                    )
                else:
                    nc.gpsimd.dma_start(
                        sc_sb[:, 0 : 8 * npc],
                        a2a_out[0].rearrange("(ue p) -> p ue", p=128),
                    )
            if nrem > 0:
                nc.vector.tensor_copy(sc_sb[:, 8 * npc :], s_all[:, 8 * npc :])

            ssum = sb.tile([128, 1], F32)
            nc.vector.tensor_reduce(
                ssum[:], sc_sb[:], axis=mybir.AxisListType.X, op=mybir.AluOpType.add
            )
            rcp = sb.tile([128, 1], F32)
            nc.vector.reciprocal(rcp[:], ssum[:])

            # ---- pooling: psum = sum_u diag(s_u) @ dah_u; out = psum/ssum --
            pool_ps = ps.tile([128, D], F32, tag="acc", bufs=4)
            for u in range(n_active):
                dw = sb.tile([128, 128], BF, name=f"dw{u}", tag="dw", bufs=2)
                nc.vector.tensor_scalar(
                    dw[:],
                    ident_sb[:],
                    sc_sb[:, u : u + 1],
                    None,
                    op0=mybir.AluOpType.mult,
                )
                for nb in range(2):
                    nc.tensor.matmul(
                        pool_ps[:, nb * 512 : (nb + 1) * 512],
                        dw[:],
                        dahp_sb[:, u, nb * 512 : (nb + 1) * 512],
                        start=(u == 0),
                        stop=(u == n_active - 1),
                    )

            out_sb = sb.tile([128, D], F32)
            nc.vector.tensor_scalar(
                out_sb[:], pool_ps[:], rcp[:], None, op0=mybir.AluOpType.mult
            )
            nc.sync.dma_start(out_p[:], out_sb[:])
            nc.sync.dma_start(scores_p[:], sc_sb[:])

    nc.compile()
    return nc


def _prep_inputs(dah, ih, qw, kw, kb, n_active):
    """Host-side shard + layout prep. Returns in_maps for 8 cores."""
    kt_local = os.environ.get("BASS_KT_LOCAL", "1") == "1"
    npc = n_active // NCORES
    nrem = n_active % NCORES

    ihT = np.ascontiguousarray(ih.T).astype(BF16)                   # [D, B]
    kwT = np.ascontiguousarray(kw.T).astype(BF16)                   # [D, D]
    biasr = np.ascontiguousarray(kb.reshape(8, 128).T).astype(np.float32)
    identity = np.eye(128, dtype=BF16)
    zeros4k = np.zeros((128, 8), dtype=np.float32)
    # qwT[u] = query_weight[u].T  -> [qd, d]
    qwT = np.ascontiguousarray(
        np.transpose(qw[:n_active], (0, 2, 1))
    ).astype(BF16)
    dah_bf = dah[:, :n_active, :].astype(BF16)                      # [B, na, D]

    in_maps = []
    for c in range(NCORES):
        m = {
            "ihT": np.ascontiguousarray(ihT[:, c * PB : (c + 1) * PB]),
            "kwT": kwT,
            "biasr": biasr,
            "ident": identity,
            "zeros4k": zeros4k,
        }
        if kt_local:
            m["ihTf"] = ihT
        if npc > 0:
            m["qwf"] = np.ascontiguousarray(qwT[npc * c : npc * (c + 1)])
        for r in range(nrem):
            m[f"qwr{r}"] = qwT[8 * npc + r]
        # score pack
        slots = np.empty((n_active, PB, D), dtype=BF16)
        for i in range(npc):
            u = npc * c + i
            for bt in range(NCORES):
                slots[i * 8 + bt] = dah_bf[bt * PB : (bt + 1) * PB, u, :]
        for r in range(nrem):
            slots[8 * npc + r] = dah_bf[c * PB : (c + 1) * PB, 8 * npc + r, :]
        m["dahs"] = slots
        # pool pack
        m["dahp"] = np.ascontiguousarray(
            np.transpose(dah_bf[c * PB : (c + 1) * PB], (1, 0, 2))
        )
        in_maps.append(m)
    return in_maps


def kernel(domain_aware_h, input_h, t, query_weight, key_weight, key_bias):
    dah = np.asarray(domain_aware_h, dtype=np.float32)
    ih = np.asarray(input_h, dtype=np.float32)
    qw = np.asarray(query_weight, dtype=np.float32)
    kw = np.asarray(key_weight, dtype=np.float32)
    kb = np.asarray(key_bias, dtype=np.float32)
    t_val = int(np.asarray(t))
    n_active = max(1, min(t_val + 1, U))

    mode = os.environ.get("BASS_MODE", "dp")
    key = (mode, n_active)
    if key not in _BUILD_CACHE:
        _BUILD_CACHE[key] = (
            _build_dp(n_active) if mode == "dp" else _build(n_active)
        )
    nc = _BUILD_CACHE[key]

    if mode == "dp":
        in_maps = _prep_inputs_dp(dah, ih, qw, kw, kb, n_active)
    else:
        in_maps = _prep_inputs(dah, ih, qw, kw, kb, n_active)
    res = run_bass_kernel_spmd(nc, in_maps, list(range(NCORES))).results

    out = np.concatenate([res[c]["out"] for c in range(NCORES)], axis=0)
    scores = np.zeros((B, U), dtype=np.float32)
    for c in range(NCORES):
        scores[c * PB : (c + 1) * PB, :n_active] = res[c]["scores"]
    return out, scores
